# revision 8
# baseline (speedup 1.0000x reference)
"""Trainium2 Bass kernel for nn_CrossAttention (B=4, C=256, N=64*64=4096, CQK=32).

Reference computation:
    q = Wq @ xf + bq          [B, N, 32]
    k = Wk @ yf + bk          [B, 32, N]
    v = Wv @ yf + bv          [B, 256, N]
    attn = softmax(q @ k)     [B, N, N]
    out = gamma * (v @ attn^T) + x

Sharding: 8 cores = batch(4) x query-half(2). Each core owns 2048 query
positions of one sample and all 4096 keys of that sample.

Optimizations over the 135us baseline:
  - Projections use fp8 DoubleRow matmuls (x/y/W* pre-cast to fp8 on host,
    channel dim folded to [128, 2, .]): one MM per tile instead of two.
  - bk is dropped (a key-bias is constant along each softmax row); gamma*bv
    is folded into the residual x on the host (sum(attn)==1 after
    normalization).
  - kT/qT are built with the projection weights replicated 4x along their
    output dim, so the K=32 contraction fills all 128 partitions (energy
    comes out 4x too large; folded into the Wq scale). Energy PSUM tiles
    span 2 banks so one instruction converts FD=1024 elements.
  - The softmax exp is split across TWO engines, strictly alternating per
    energy tile so neither engine gates the PE:
      * ScalarE: true exp via activation (scale=ln2/8, bias=-ln2/2) -> fp8.
      * VectorE: Schraudolph bit trick - uint8 bits = max(E*8/ln2 + 52, 0)
        IS the fp8e4m3 encoding of exp(E)/sqrt(2).
    (Energy is pre-scaled by (8/ln2)/4, folded into Wq/bq on the host.)
    Both paths emit exp(E)/sqrt(2); the constant cancels in softmax.
  - AV matmuls: fp8 DoubleRow, vaug [128,2,272]; the augmentation column
    holds 1/gamma, so the accumulated denominator column is den/gamma and
    its reciprocal is gamma/den.
  - The output stays in [query, channel] layout: per n-chunk the whole
    drain is reciprocal + ONE fused scalar_tensor_tensor
    (pout * (gamma/den)) + x_residual, then a direct DMA out. The host
    un-transposes during assembly (free in numpy). No PE transposes.
"""

import contextlib
import math

import numpy as np

import concourse.mybir as mybir
import concourse.tile as tile
from concourse import bacc
from concourse.bass_utils import run_bass_kernel_spmd

F32 = mybir.dt.float32
F8 = mybir.dt.float8e4
U8 = mybir.dt.uint8
BF16 = mybir.dt.bfloat16
AFT = mybir.ActivationFunctionType
ALU = mybir.AluOpType
DR = mybir.MatmulPerfMode.DoubleRow

B = 4
C = 256
CQK = 32
N = 4096  # 64 * 64
NCORES = 8
NLOC = N // 2  # 2048 queries per core
CCH = C // 128  # 2 channel chunks
MC = N // 128  # 32 key chunks
NQ = 4  # query quarters per core
QW = NLOC // NQ  # 512
NCHUNKS = NLOC // 128  # 16 query chunks per core
VW = 272  # vaug width: 256 v channels + denominator col + pad (step%16==0)

SC = 8.0 / math.log(2.0)  # 11.5416; SC/4 folded into Wq/bq on the host
EXP_SCALE = 1.0 / SC
EXP_BIAS = -0.5 * math.log(2.0)  # both exp paths emit exp(E)/sqrt(2)
BITS_OFF = 52.0  # fp8 bits = max(E*SC + 52, 0): 2^((b-56)/8) = exp(E)/sqrt2


def _trace_kernel(ctx, tc, x_d, xq_d, y_d, wq_d, wk_d, wv_d, bq_d, g_d, out_d):
    nc = tc.nc

    const = ctx.enter_context(tc.tile_pool(name="const", bufs=1))
    big = ctx.enter_context(tc.tile_pool(name="big", bufs=1))
    vaugp = ctx.enter_context(tc.tile_pool(name="vaugp", bufs=MC // 2))
    expp = ctx.enter_context(tc.tile_pool(name="expp", bufs=4))
    finp = ctx.enter_context(tc.tile_pool(name="finp", bufs=3))
    smallp = ctx.enter_context(tc.tile_pool(name="smallp", bufs=6))
    # PSUM budget (8 banks): 2 double-bank energy tiles + 4 x 1-bank pouts
    poutp = ctx.enter_context(tc.tile_pool(name="poutp", bufs=4, space="PSUM"))
    pep = ctx.enter_context(tc.tile_pool(name="pep", bufs=2, space="PSUM"))

    # ---- constant / weight loads (pre-cast to fp8 on host) ----
    wq_b = const.tile([128, CCH, 128], F8, tag="wq_b")
    nc.sync.dma_start(out=wq_b, in_=wq_d.ap())
    wk_b = const.tile([128, CCH, 128], F8, tag="wk_b")
    nc.sync.dma_start(out=wk_b, in_=wk_d.ap())
    wv_b = const.tile([128, CCH, C], F8, tag="wv_b")
    nc.sync.dma_start(out=wv_b, in_=wv_d.ap())
    bq_sb = const.tile([128, 1], F32, tag="bq_sb")
    nc.sync.dma_start(out=bq_sb, in_=bq_d.ap())
    g_sb = const.tile([128, 1], F32, tag="g_sb")
    nc.sync.dma_start(out=g_sb, in_=g_d.ap())
    ebias_sb = const.tile([128, 1], F32, tag="ebias_sb")
    nc.vector.memset(ebias_sb, EXP_BIAS)
    # vaug augmentation column = 1/gamma -> denominator column accumulates
    # den/gamma, so its reciprocal is the fused normalize+gamma scale.
    rg_sb = const.tile([128, 1], F32, tag="rg_sb")
    nc.vector.reciprocal(rg_sb, g_sb)
    onep_sb = const.tile([128, CCH, VW - C], F8, tag="onep_sb")
    nc.vector.memset(onep_sb, 0.0)
    for j in range(CCH):
        nc.vector.tensor_copy(onep_sb[:, j, 0:1], rg_sb)
    # get the exp table load out of the way during the DMA phase; Copy is
    # filler in every table set so later Copy activations don't reload.
    warm = const.tile([1, 1], F32, tag="warm")
    nc.scalar.activation(warm, bq_sb[0:1, :], AFT.Exp)

    # ---- activations in: xq first (q projection only needs it), y in two
    # big chunks on separate rings (fewer, larger DMA descriptors).
    xq_b = big.tile([128, CCH, NLOC], F8, tag="xq_b")
    for j in range(CCH):
        nc.gpsimd.dma_start(out=xq_b[:, j, :], in_=xq_d.ap()[:, j, :])
    y_b = big.tile([128, CCH, N], F8, tag="y_b")
    y_queues = [nc.scalar, nc.sync]
    for j in range(CCH):
        y_queues[j].dma_start(out=y_b[:, j, :], in_=y_d.ap()[:, j, :])
    # residual x (+ gamma*bv), [n, e] layout: tile [128, NCHUNKS, C]
    xr_sb = big.tile([128, NCHUNKS, C], F32, tag="xr_sb")

    # ---- q projection (replicated 4x across partition groups) ----
    qT4 = big.tile([128, NLOC], BF16, tag="qT4")
    for nt in range(NLOC // QW):
        pq = pep.tile([128, QW], F32, tag="pe", name=f"pq{nt}")
        nc.tensor.matmul(
            pq,
            lhsT=wq_b,
            rhs=xq_b[:, :, nt * QW : (nt + 1) * QW],
            start=True,
            stop=True,
            perf_mode=DR,
        )
        nc.vector.tensor_scalar_add(qT4[:, nt * QW : (nt + 1) * QW], pq, bq_sb)

    # ---- k projection ----
    kT4 = big.tile([128, N], BF16, tag="kT4")
    for nt in range(N // QW):
        pk = pep.tile([128, QW], F32, tag="pe", name=f"pk{nt}")
        nc.tensor.matmul(
            pk,
            lhsT=wk_b,
            rhs=y_b[:, :, nt * QW : (nt + 1) * QW],
            start=True,
            stop=True,
            perf_mode=DR,
        )
        # no bias: a k-bias is constant per query row and cancels in softmax
        nc.scalar.activation(kT4[:, nt * QW : (nt + 1) * QW], pk, AFT.Copy)

    # ---- vaugT fp8 pair tiles [128, 2, VW] for DoubleRow AV ----
    vaug = []
    for t in range(MC // 2):
        va = vaugp.tile([128, 2, VW], F8, tag="vaug", name=f"vaug{t}")
        for j in range(2):
            mc = 2 * t + j
            pv = pep.tile([128, C], F32, tag="pe", name=f"pv{mc}")
            nc.tensor.matmul(
                pv,
                lhsT=y_b[:, :, mc * 128 : (mc + 1) * 128],
                rhs=wv_b,
                start=True,
                stop=True,
                perf_mode=DR,
            )
            if mc % 2 == 0:
                nc.scalar.activation(va[:, j, :C], pv, AFT.Copy)
            else:
                nc.vector.tensor_copy(va[:, j, :C], pv)
            nc.vector.tensor_copy(va[:, j, C:VW], onep_sb[:, j, :])
        vaug.append(va)

    # residual stream: after all critical-path DMAs on the sync queue
    nc.sync.dma_start(out=xr_sb, in_=x_d.ap())

    # ---- attention quarters ----
    for qt in range(NQ):
        nsl = slice(qt * QW, (qt + 1) * QW)
        pouts = [
            poutp.tile([128, VW], F32, tag="pout", name=f"pout{qt}_{i}")
            for i in range(4)
        ]

        def do_av(exs, g):
            # AV for the two exp pair-tiles of group g (pairs 2g, 2g+1)
            for jj, ex in enumerate(exs):
                p = 2 * g + jj
                for ncc in range(4):
                    nc.tensor.matmul(
                        pouts[ncc],
                        lhsT=ex[:, :, ncc * 128 : (ncc + 1) * 128],
                        rhs=vaug[p],
                        start=(p == 0),
                        stop=(p == MC // 2 - 1),
                        perf_mode=DR,
                    )

        prev = None
        for g in range(8):
            # 4 energy MMs (key chunks 4g..4g+3) into two 2-bank PSUM tiles
            pes = [
                pep.tile([128, 2, QW], F32, tag="pe", name=f"pe{qt}_{g}_{h}")
                for h in range(2)
            ]
            for i in range(4):
                m = 4 * g + i
                nc.tensor.matmul(
                    pes[i // 2][:, i % 2, :],
                    lhsT=kT4[:, m * 128 : (m + 1) * 128],
                    rhs=qT4[:, nsl],
                    start=True,
                    stop=True,
                )
            exs = []
            for h in range(2):
                ex = expp.tile(
                    [128, 2, QW], F8, tag="exp", name=f"ex{qt}_{g}_{h}"
                )
                if h == 0:
                    # Schraudolph: uint8 bits of max(E*SC+52, 0) ARE the
                    # fp8e4m3 encoding of exp(E)/sqrt(2)
                    nc.vector.tensor_scalar(
                        out=ex.bitcast(U8),
                        in0=pes[h],
                        scalar1=BITS_OFF,
                        scalar2=0.0,
                        op0=ALU.add,
                        op1=ALU.max,
                    )
                else:
                    nc.scalar.activation(
                        ex, pes[h], AFT.Exp, bias=ebias_sb, scale=EXP_SCALE
                    )
                exs.append(ex)
            if prev is not None:
                do_av(*prev)
            prev = (exs, g)
        do_av(*prev)

        # drain: normalize+gamma on ScalarE (frees the pout bank fast),
        # residual add on GpSimd, one batched DMA per quarter; output stays
        # in [query, channel] layout (host un-transposes during assembly)
        finq = finp.tile([128, 4, C], F32, tag="fin", name=f"finq{qt}")
        for ncc in range(4):
            po = pouts[ncc]
            rec = smallp.tile([128, 1], F32, tag="rec", name=f"rec{qt}_{ncc}")
            nc.vector.reciprocal(rec, po[:, C : C + 1])
            onn = finp.tile([128, C], F32, tag="onn", name=f"onn{qt}_{ncc}")
            nc.scalar.activation(onn, po[:, :C], AFT.Copy, scale=rec)
            nc.gpsimd.tensor_add(
                finq[:, ncc, :], onn, xr_sb[:, 4 * qt + ncc, :]
            )
        nc.sync.dma_start(out=out_d.ap()[:, qt, :, :], in_=finq)


_PROGRAM_CACHE = {}


def _get_program():
    if "nc" in _PROGRAM_CACHE:
        return _PROGRAM_CACHE["nc"]
    nc = bacc.Bacc("TRN2", target_bir_lowering=False, debug=False)
    x_d = nc.dram_tensor("x_loc", [128, NCHUNKS, C], F32, kind="ExternalInput")
    xq_d = nc.dram_tensor("x_q8", [128, CCH, NLOC], F8, kind="ExternalInput")
    y_d = nc.dram_tensor("y_q8", [128, CCH, N], F8, kind="ExternalInput")
    wq_d = nc.dram_tensor("wq4", [128, CCH, 128], F8, kind="ExternalInput")
    wk_d = nc.dram_tensor("wk4", [128, CCH, 128], F8, kind="ExternalInput")
    wv_d = nc.dram_tensor("wv_t", [128, CCH, C], F8, kind="ExternalInput")
    bq_d = nc.dram_tensor("bq4", [128, 1], F32, kind="ExternalInput")
    g_d = nc.dram_tensor("gamma_b", [128, 1], F32, kind="ExternalInput")
    out_d = nc.dram_tensor(
        "out_loc", [128, NQ, 4, C], F32, kind="ExternalOutput"
    )
    with tile.TileContext(nc) as tc, contextlib.ExitStack() as ctx:
        _trace_kernel(ctx, tc, x_d, xq_d, y_d, wq_d, wk_d, wv_d, bq_d, g_d, out_d)
    nc.compile()
    _PROGRAM_CACHE["nc"] = nc
    return nc


def _make_in_maps(inputs):
    import ml_dtypes

    F8NP = ml_dtypes.float8_e4m3
    x = np.ascontiguousarray(inputs["x"], dtype=np.float32).reshape(B, C, N)
    y = np.asarray(inputs["y"], np.float32).reshape(B, C, N)
    gamma = float(np.asarray(inputs["gamma"]).reshape(-1)[0])
    bv = np.asarray(inputs["bv"], np.float32)
    # residual carries x + gamma*bv (sum of normalized attn weights == 1)
    xr = x + gamma * bv[None, :, None]
    # fp8 DoubleRow layouts: channel c -> (partition c%128, ktile c//128)
    y8 = np.ascontiguousarray(
        y.reshape(B, CCH, 128, N).transpose(0, 2, 1, 3).astype(F8NP)
    )
    x8 = np.ascontiguousarray(
        x.reshape(B, CCH, 128, N).transpose(0, 2, 1, 3).astype(F8NP)
    )

    def wlayout(w):  # [out_dim, C] -> [128, CCH, out_dim] fp8
        return np.ascontiguousarray(
            w.T.reshape(CCH, 128, w.shape[0]).transpose(1, 0, 2).astype(F8NP)
        )

    wq4 = wlayout(np.tile(np.asarray(inputs["Wq"], np.float32) * (SC / 4), (4, 1)))
    wk4 = wlayout(np.tile(np.asarray(inputs["Wk"], np.float32), (4, 1)))
    wv_t = wlayout(np.asarray(inputs["Wv"], np.float32))
    bq4 = np.ascontiguousarray(
        np.tile(np.asarray(inputs["bq"], np.float32) * (SC / 4), 4).reshape(128, 1)
    )
    gamma_b = np.full((128, 1), gamma, np.float32)

    in_maps = []
    for core in range(NCORES):
        b, h = divmod(core, 2)
        nsl = slice(h * NLOC, (h + 1) * NLOC)
        # residual in [n%128, n-chunk, channel] layout (one big DMA)
        x_loc = np.ascontiguousarray(
            xr[b, :, nsl].T.reshape(NCHUNKS, 128, C).transpose(1, 0, 2)
        )
        in_maps.append(
            {
                "x_loc": x_loc,
                "x_q8": np.ascontiguousarray(x8[b][:, :, nsl]),
                "y_q8": y8[b],
                "wq4": wq4,
                "wk4": wk4,
                "wv_t": wv_t,
                "bq4": bq4,
                "gamma_b": gamma_b,
            }
        )
    return in_maps


def _assemble(results):
    out = np.empty((B, C, N), np.float32)
    for core in range(NCORES):
        b, h = divmod(core, 2)
        r = results[core]["out_loc"].reshape(128, NCHUNKS, C)
        out[b, :, h * NLOC : (h + 1) * NLOC] = (
            r.transpose(1, 0, 2).reshape(NLOC, C).T
        )
    return out.reshape(B, C, 64, 64)


def run(inputs, trace=False, **kwargs):
    """Run the kernel; returns (full_output, BassKernelResults)."""
    nc = _get_program()
    in_maps = _make_in_maps(inputs)
    res = run_bass_kernel_spmd(
        nc, in_maps, core_ids=list(range(NCORES)), trace=trace, **kwargs
    )
    return _assemble(res.results), res


def kernel(**inputs) -> np.ndarray:
    out, _ = run(inputs, trace=False)
    return out


# revision 10
# speedup vs baseline: 1.0240x; 1.0240x over previous
"""Trainium2 Bass kernel for nn_CrossAttention (B=4, C=256, N=64*64=4096, CQK=32).

Reference computation:
    q = Wq @ xf + bq          [B, N, 32]
    k = Wk @ yf + bk          [B, 32, N]
    v = Wv @ yf + bv          [B, 256, N]
    attn = softmax(q @ k)     [B, N, N]
    out = gamma * (v @ attn^T) + x

Sharding: 8 cores = batch(4) x query-half(2). Each core owns 2048 query
positions of one sample and all 4096 keys of that sample.

Optimizations over the 135us baseline:
  - Projections use fp8 DoubleRow matmuls (x/y/W* pre-cast to fp8 on host,
    channel dim folded to [128, 2, .]): one MM per tile instead of two.
  - bk is dropped (a key-bias is constant along each softmax row); gamma*bv
    is folded into the residual x on the host (sum(attn)==1 after
    normalization).
  - kT/qT are built with the projection weights replicated 4x along their
    output dim, so the K=32 contraction fills all 128 partitions (energy
    comes out 4x too large; folded into the Wq scale). Energy PSUM tiles
    span 2 banks so one instruction converts FD=1024 elements.
  - The softmax exp is split across TWO engines, strictly alternating per
    energy tile so neither engine gates the PE:
      * ScalarE: true exp via activation (scale=ln2/8, bias=-ln2/2) -> fp8.
      * VectorE: Schraudolph bit trick - uint8 bits = max(E*8/ln2 + 52, 0)
        IS the fp8e4m3 encoding of exp(E)/sqrt(2).
    (Energy is pre-scaled by (8/ln2)/4, folded into Wq/bq on the host.)
    Both paths emit exp(E)/sqrt(2); the constant cancels in softmax.
  - AV matmuls: fp8 DoubleRow, vaug [128,2,272]; the augmentation column
    holds 1/gamma, so the accumulated denominator column is den/gamma and
    its reciprocal is gamma/den.
  - The output stays in [query, channel] layout: per n-chunk the whole
    drain is reciprocal + ONE fused scalar_tensor_tensor
    (pout * (gamma/den)) + x_residual, then a direct DMA out. The host
    un-transposes during assembly (free in numpy). No PE transposes.
"""

import contextlib
import math

import numpy as np

import concourse.mybir as mybir
import concourse.tile as tile
from concourse import bacc
from concourse.bass_utils import run_bass_kernel_spmd

F32 = mybir.dt.float32
F8 = mybir.dt.float8e4
U8 = mybir.dt.uint8
BF16 = mybir.dt.bfloat16
AFT = mybir.ActivationFunctionType
ALU = mybir.AluOpType
DR = mybir.MatmulPerfMode.DoubleRow

B = 4
C = 256
CQK = 32
N = 4096  # 64 * 64
NCORES = 8
NLOC = N // 2  # 2048 queries per core
CCH = C // 128  # 2 channel chunks
MC = N // 128  # 32 key chunks
NQ = 4  # query quarters per core
QW = NLOC // NQ  # 512
NCHUNKS = NLOC // 128  # 16 query chunks per core
VW = 272  # vaug width: 256 v channels + denominator col + pad (step%16==0)

SC = 8.0 / math.log(2.0)  # 11.5416; SC/4 folded into Wq/bq on the host
EXP_SCALE = 1.0 / SC
EXP_BIAS = -0.5 * math.log(2.0)  # both exp paths emit exp(E)/sqrt(2)
BITS_OFF = 52.0  # fp8 bits = max(E*SC + 52, 0): 2^((b-56)/8) = exp(E)/sqrt2


def _trace_kernel(ctx, tc, x_d, xq_d, y_d, wq_d, wk_d, wv_d, bq_d, g_d, out_d):
    nc = tc.nc

    const = ctx.enter_context(tc.tile_pool(name="const", bufs=1))
    big = ctx.enter_context(tc.tile_pool(name="big", bufs=1))
    vaugp = ctx.enter_context(tc.tile_pool(name="vaugp", bufs=MC // 2))
    expp = ctx.enter_context(tc.tile_pool(name="expp", bufs=4))
    finp = ctx.enter_context(tc.tile_pool(name="finp", bufs=3))
    smallp = ctx.enter_context(tc.tile_pool(name="smallp", bufs=6))
    # PSUM budget (8 banks): 2 double-bank energy tiles + 4 x 1-bank pouts
    poutp = ctx.enter_context(tc.tile_pool(name="poutp", bufs=4, space="PSUM"))
    pep = ctx.enter_context(tc.tile_pool(name="pep", bufs=2, space="PSUM"))

    # ---- constant / weight loads (pre-cast to fp8 on host) ----
    wq_b = const.tile([128, CCH, 128], F8, tag="wq_b")
    nc.sync.dma_start(out=wq_b, in_=wq_d.ap())
    wk_b = const.tile([128, CCH, 128], F8, tag="wk_b")
    nc.sync.dma_start(out=wk_b, in_=wk_d.ap())
    wv_b = const.tile([128, CCH, C], F8, tag="wv_b")
    nc.sync.dma_start(out=wv_b, in_=wv_d.ap())
    bq_sb = const.tile([128, 1], F32, tag="bq_sb")
    nc.sync.dma_start(out=bq_sb, in_=bq_d.ap())
    g_sb = const.tile([128, 1], F32, tag="g_sb")
    nc.sync.dma_start(out=g_sb, in_=g_d.ap())
    ebias_sb = const.tile([128, 1], F32, tag="ebias_sb")
    nc.vector.memset(ebias_sb, EXP_BIAS)
    # vaug augmentation column = 1/gamma -> denominator column accumulates
    # den/gamma, so its reciprocal is the fused normalize+gamma scale.
    rg_sb = const.tile([128, 1], F32, tag="rg_sb")
    nc.vector.reciprocal(rg_sb, g_sb)
    onep_sb = const.tile([128, CCH, VW - C], F8, tag="onep_sb")
    nc.vector.memset(onep_sb, 0.0)
    for j in range(CCH):
        nc.vector.tensor_copy(onep_sb[:, j, 0:1], rg_sb)
    # get the exp table load out of the way during the DMA phase; Copy is
    # filler in every table set so later Copy activations don't reload.
    warm = const.tile([1, 1], F32, tag="warm")
    nc.scalar.activation(warm, bq_sb[0:1, :], AFT.Exp)

    # ---- activations in: xq first (q projection only needs it), y in two
    # big chunks on separate rings (fewer, larger DMA descriptors).
    xq_b = big.tile([128, CCH, NLOC], F8, tag="xq_b")
    for d in range(2):
        sl = slice(d * (NLOC // 2), (d + 1) * (NLOC // 2))
        nc.gpsimd.dma_start(out=xq_b[:, :, sl], in_=xq_d.ap()[d])
    y_b = big.tile([128, CCH, N], F8, tag="y_b")
    y_queues = [nc.scalar, nc.sync]
    for d in range(4):
        sl = slice(d * (N // 4), (d + 1) * (N // 4))
        y_queues[d % 2].dma_start(out=y_b[:, :, sl], in_=y_d.ap()[d])
    # residual x (+ gamma*bv), [n, e] layout: tile [128, NCHUNKS, C]
    xr_sb = big.tile([128, NCHUNKS, C], F32, tag="xr_sb")

    # ---- q projection (replicated 4x across partition groups) ----
    qT4 = big.tile([128, NLOC], BF16, tag="qT4")
    pools = [pep, poutp]
    ptags = ["pe", "pout"]
    for nt in range(NLOC // QW):
        pq = pools[nt % 2].tile([128, QW], F32, tag=ptags[nt % 2], name=f"pq{nt}")
        nc.tensor.matmul(
            pq,
            lhsT=wq_b,
            rhs=xq_b[:, :, nt * QW : (nt + 1) * QW],
            start=True,
            stop=True,
            perf_mode=DR,
        )
        nc.vector.tensor_scalar_add(qT4[:, nt * QW : (nt + 1) * QW], pq, bq_sb)

    # ---- k projection ----
    kT4 = big.tile([128, N], BF16, tag="kT4")
    for nt in range(N // QW):
        pk = pools[nt % 2].tile([128, QW], F32, tag=ptags[nt % 2], name=f"pk{nt}")
        nc.tensor.matmul(
            pk,
            lhsT=wk_b,
            rhs=y_b[:, :, nt * QW : (nt + 1) * QW],
            start=True,
            stop=True,
            perf_mode=DR,
        )
        # no bias: a k-bias is constant per query row and cancels in softmax
        if nt % 2 == 0:
            nc.scalar.activation(kT4[:, nt * QW : (nt + 1) * QW], pk, AFT.Copy)
        else:
            nc.vector.tensor_copy(kT4[:, nt * QW : (nt + 1) * QW], pk)

    # ---- vaugT fp8 pair tiles [128, 2, VW] for DoubleRow AV ----
    vaug = []
    for t in range(MC // 2):
        va = vaugp.tile([128, 2, VW], F8, tag="vaug", name=f"vaug{t}")
        for j in range(2):
            mc = 2 * t + j
            pv = pools[mc % 2].tile(
                [128, C], F32, tag=ptags[mc % 2], name=f"pv{mc}"
            )
            nc.tensor.matmul(
                pv,
                lhsT=y_b[:, :, mc * 128 : (mc + 1) * 128],
                rhs=wv_b,
                start=True,
                stop=True,
                perf_mode=DR,
            )
            if mc % 2 == 0:
                nc.scalar.activation(va[:, j, :C], pv, AFT.Copy)
            else:
                nc.vector.tensor_copy(va[:, j, :C], pv)
            nc.vector.tensor_copy(va[:, j, C:VW], onep_sb[:, j, :])
        vaug.append(va)

    # residual stream: after all critical-path DMAs on the sync queue
    nc.sync.dma_start(out=xr_sb, in_=x_d.ap())

    # ---- attention quarters ----
    def emit_drain(pouts, qt):
        # normalize+gamma on ScalarE (frees the pout bank fast), residual
        # add on GpSimd, one batched DMA per quarter; output stays in
        # [query, channel] layout (host un-transposes during assembly)
        finq = finp.tile([128, 4, C], F32, tag="fin", name=f"finq{qt}")
        for ncc in range(4):
            po = pouts[ncc]
            rec = smallp.tile([128, 1], F32, tag="rec", name=f"rec{qt}_{ncc}")
            nc.vector.reciprocal(rec, po[:, C : C + 1])
            onn = finp.tile([128, C], F32, tag="onn", name=f"onn{qt}_{ncc}")
            nc.scalar.activation(onn, po[:, :C], AFT.Copy, scale=rec)
            nc.gpsimd.tensor_add(
                finq[:, ncc, :], onn, xr_sb[:, 4 * qt + ncc, :]
            )
        nc.sync.dma_start(out=out_d.ap()[:, qt, :, :], in_=finq)

    pending_drain = None
    for qt in range(NQ):
        nsl = slice(qt * QW, (qt + 1) * QW)
        pouts = [
            poutp.tile([128, VW], F32, tag="pout", name=f"pout{qt}_{i}")
            for i in range(4)
        ]

        def do_av(exs, g):
            # AV for the two exp pair-tiles of group g (pairs 2g, 2g+1)
            for jj, ex in enumerate(exs):
                p = 2 * g + jj
                for ncc in range(4):
                    nc.tensor.matmul(
                        pouts[ncc],
                        lhsT=ex[:, :, ncc * 128 : (ncc + 1) * 128],
                        rhs=vaug[p],
                        start=(p == 0),
                        stop=(p == MC // 2 - 1),
                        perf_mode=DR,
                    )

        prev = None
        for g in range(8):
            # 4 energy MMs (key chunks 4g..4g+3) into two 2-bank PSUM tiles
            pes = [
                pep.tile([128, 2, QW], F32, tag="pe", name=f"pe{qt}_{g}_{h}")
                for h in range(2)
            ]
            for i in range(4):
                m = 4 * g + i
                nc.tensor.matmul(
                    pes[i // 2][:, i % 2, :],
                    lhsT=kT4[:, m * 128 : (m + 1) * 128],
                    rhs=qT4[:, nsl],
                    start=True,
                    stop=True,
                )
            exs = []
            for h in range(2):
                ex = expp.tile(
                    [128, 2, QW], F8, tag="exp", name=f"ex{qt}_{g}_{h}"
                )
                if h == 0:
                    # Schraudolph: uint8 bits of max(E*SC+52, 0) ARE the
                    # fp8e4m3 encoding of exp(E)/sqrt(2)
                    nc.vector.tensor_scalar(
                        out=ex.bitcast(U8),
                        in0=pes[h],
                        scalar1=BITS_OFF,
                        scalar2=0.0,
                        op0=ALU.add,
                        op1=ALU.max,
                    )
                else:
                    # two FD=512 activations so the first starts as soon as
                    # the third energy MM lands (the PE never waits on exp)
                    for j in range(2):
                        nc.scalar.activation(
                            ex[:, j, :],
                            pes[h][:, j, :],
                            AFT.Exp,
                            bias=ebias_sb,
                            scale=EXP_SCALE,
                        )
                exs.append(ex)
            if g == 0 and pending_drain is not None:
                # issue the previous quarter's drain AFTER this quarter's
                # first exps, so the boundary exp isn't queued behind it
                emit_drain(*pending_drain)
            if prev is not None:
                do_av(*prev)
            prev = (exs, g)
        do_av(*prev)
        pending_drain = (pouts, qt)
    emit_drain(*pending_drain)


_PROGRAM_CACHE = {}


def _get_program():
    if "nc" in _PROGRAM_CACHE:
        return _PROGRAM_CACHE["nc"]
    nc = bacc.Bacc("TRN2", target_bir_lowering=False, debug=False)
    x_d = nc.dram_tensor("x_loc", [128, NCHUNKS, C], F32, kind="ExternalInput")
    xq_d = nc.dram_tensor(
        "x_q8", [2, 128, CCH, NLOC // 2], F8, kind="ExternalInput"
    )
    y_d = nc.dram_tensor("y_q8", [4, 128, CCH, N // 4], F8, kind="ExternalInput")
    wq_d = nc.dram_tensor("wq4", [128, CCH, 128], F8, kind="ExternalInput")
    wk_d = nc.dram_tensor("wk4", [128, CCH, 128], F8, kind="ExternalInput")
    wv_d = nc.dram_tensor("wv_t", [128, CCH, C], F8, kind="ExternalInput")
    bq_d = nc.dram_tensor("bq4", [128, 1], F32, kind="ExternalInput")
    g_d = nc.dram_tensor("gamma_b", [128, 1], F32, kind="ExternalInput")
    out_d = nc.dram_tensor(
        "out_loc", [128, NQ, 4, C], F32, kind="ExternalOutput"
    )
    with tile.TileContext(nc) as tc, contextlib.ExitStack() as ctx:
        _trace_kernel(ctx, tc, x_d, xq_d, y_d, wq_d, wk_d, wv_d, bq_d, g_d, out_d)
    nc.compile()
    _PROGRAM_CACHE["nc"] = nc
    return nc


def _make_in_maps(inputs):
    import ml_dtypes

    F8NP = ml_dtypes.float8_e4m3
    x = np.ascontiguousarray(inputs["x"], dtype=np.float32).reshape(B, C, N)
    y = np.asarray(inputs["y"], np.float32).reshape(B, C, N)
    gamma = float(np.asarray(inputs["gamma"]).reshape(-1)[0])
    bv = np.asarray(inputs["bv"], np.float32)
    # residual carries x + gamma*bv (sum of normalized attn weights == 1)
    xr = x + gamma * bv[None, :, None]
    # fp8 DoubleRow layouts: channel c -> (partition c%128, ktile c//128);
    # column-chunked outer dim so each DMA chunk is dram-contiguous
    y8 = np.ascontiguousarray(
        y.reshape(B, CCH, 128, 4, N // 4).transpose(0, 3, 2, 1, 4).astype(F8NP)
    )
    x8 = np.ascontiguousarray(
        x.reshape(B, CCH, 128, N).transpose(0, 2, 1, 3).astype(F8NP)
    )

    def wlayout(w):  # [out_dim, C] -> [128, CCH, out_dim] fp8
        return np.ascontiguousarray(
            w.T.reshape(CCH, 128, w.shape[0]).transpose(1, 0, 2).astype(F8NP)
        )

    wq4 = wlayout(np.tile(np.asarray(inputs["Wq"], np.float32) * (SC / 4), (4, 1)))
    wk4 = wlayout(np.tile(np.asarray(inputs["Wk"], np.float32), (4, 1)))
    wv_t = wlayout(np.asarray(inputs["Wv"], np.float32))
    bq4 = np.ascontiguousarray(
        np.tile(np.asarray(inputs["bq"], np.float32) * (SC / 4), 4).reshape(128, 1)
    )
    gamma_b = np.full((128, 1), gamma, np.float32)

    in_maps = []
    for core in range(NCORES):
        b, h = divmod(core, 2)
        nsl = slice(h * NLOC, (h + 1) * NLOC)
        # residual in [n%128, n-chunk, channel] layout (one big DMA)
        x_loc = np.ascontiguousarray(
            xr[b, :, nsl].T.reshape(NCHUNKS, 128, C).transpose(1, 0, 2)
        )
        xq_loc = x8[b][:, :, nsl]  # [128, CCH, NLOC]
        xq_loc = np.ascontiguousarray(
            xq_loc.reshape(128, CCH, 2, NLOC // 2).transpose(2, 0, 1, 3)
        )
        in_maps.append(
            {
                "x_loc": x_loc,
                "x_q8": xq_loc,
                "y_q8": y8[b],
                "wq4": wq4,
                "wk4": wk4,
                "wv_t": wv_t,
                "bq4": bq4,
                "gamma_b": gamma_b,
            }
        )
    return in_maps


def _assemble(results):
    out = np.empty((B, C, N), np.float32)
    for core in range(NCORES):
        b, h = divmod(core, 2)
        r = results[core]["out_loc"].reshape(128, NCHUNKS, C)
        out[b, :, h * NLOC : (h + 1) * NLOC] = (
            r.transpose(1, 0, 2).reshape(NLOC, C).T
        )
    return out.reshape(B, C, 64, 64)


def run(inputs, trace=False, **kwargs):
    """Run the kernel; returns (full_output, BassKernelResults)."""
    nc = _get_program()
    in_maps = _make_in_maps(inputs)
    res = run_bass_kernel_spmd(
        nc, in_maps, core_ids=list(range(NCORES)), trace=trace, **kwargs
    )
    return _assemble(res.results), res


def kernel(**inputs) -> np.ndarray:
    out, _ = run(inputs, trace=False)
    return out
